# revision 22
# baseline (speedup 1.0000x reference)
"""Trainium2 Bass kernel for nn_CropPrompter.

Fused resize+crop bilinear sampling of video clips:
  x[8,3,16,512,512] --(per-clip crop geometry from cam_views/resize/offsets)-->
  out[8,3,16,224,224]

Strategy (pure data parallel, 1 clip per NeuronCore, 8 cores), v2:
  * All device compute in bf16 (PSUM accumulates f32).  Host converts x to
    bf16 (halves input DMA), packs the per-view bilinear interpolation
    matrices in bf16, and upcasts the bf16 device output back to f32.
    Measured end-to-end rel err ~2.9e-3 vs the f32 reference (gate 2e-2).
  * Banded stage 1: because resize >= H=512, the source-row span of any 112
    consecutive output rows is <= 114 < 128.  The device loads, per clip, two
    data-dependent 128-row x 256-col windows of each frame (dynamic-offset
    DMA; the row/col window bases ride in as an int32 `offs` input read into
    sequencer registers), so the row interpolation for each output half is a
    SINGLE k=128 matmul instead of a k=256 accumulation pair:
      A^T[w', i-half m] = win_m^T @ RyT_m      4 matmuls @ N=112 per frame
  * Stage 2 contracts the static 256-col window (2 k-tiles):
      out[i, j]        = A^T^T @ RxT           4 matmuls @ N=224 per frame
  * bf16 matmuls run 1 cycle/row at any N (f32r needs N>=256 for full rate),
    and full-128-col bf16 weights get Fast Weight Load; LDWEIGHTS pulls
    ahead of in-flight matmuls so weight loads hide behind streaming.
  * A^T's i axis is (r, p) pair-permuted (i = 2p+r, via host permutation of
    RyT's columns) so each output-store partition holds an i row pair ->
    896 B contiguous DRAM runs (>= 512 B: no DMA read-modify-write penalty).
"""

import numpy as np
import ml_dtypes

CROP = 224
H = 512
RESIZE_MAX = 1024
WIN = 256  # static col-window width (cols [xbase, xbase+256) cover all taps)

_PROGRAM = None
TRACE = False
LAST_RESULTS = None

BF16 = ml_dtypes.bfloat16


def _coords(off, rb):
    """Replicates reference._coords in numpy float32, op-for-op."""
    i = np.arange(CROP, dtype=np.float32)
    src = (np.float32(off) + i + np.float32(0.5)) * (np.float32(H) / np.float32(rb)) - np.float32(0.5)
    src = np.maximum(src, np.float32(0.0))
    i0 = np.clip(np.floor(src).astype(np.int32), 0, H - 1)
    i1 = np.minimum(i0 + 1, H - 1)
    w = src - i0.astype(np.float32)
    return i0, i1, w


def _tap_matrix(i0, i1, w, base, K, N):
    """[K, N] f32 M with M[k, n] = bilinear weight of source row (base+k)
    for output row n (of the N-slice whose taps i0/i1/w are passed)."""
    m = np.zeros((K, N), dtype=np.float32)
    idx = np.arange(N)
    np.add.at(m, (i0 - base, idx), np.float32(1.0) - w)
    np.add.at(m, (i1 - base, idx), w)
    return m


def _split_multi_waits(nc):
    """Walrus (kernel-dev pipeline) allows only one semaphore wait per
    instruction; hoist extra waits onto standalone EventSemaphore
    instructions inserted just before, on the same engine."""
    from concourse import mybir

    n = 0
    for fn in nc.m.functions:
        for bb in fn.blocks:
            out = []
            changed = False
            for inst in bb.instructions:
                si = getattr(inst, "sync_info", None)
                waits = list(si.on_wait) if si is not None and si.on_wait else []
                if len(waits) > 1:
                    for k, w in enumerate(waits[:-1]):
                        out.append(
                            mybir.InstEventSemaphore(
                                name=f"{inst.name}-w{k}",
                                ins=[],
                                outs=[],
                                engine=inst.engine,
                                sync_info=mybir.SyncInfo(on_wait=[w], on_update=[]),
                            )
                        )
                        n += 1
                    inst.sync_info = mybir.SyncInfo(
                        on_wait=[waits[-1]], on_update=list(si.on_update or [])
                    )
                    changed = True
                out.append(inst)
            if changed:
                bb.instructions = out
    return n


def _build_program():
    from concourse import bass, mybir, tile

    f32 = mybir.dt.float32
    bf16 = mybir.dt.bfloat16
    i32 = mybir.dt.int32

    nc = bass.Bass()
    xcb = nc.dram_tensor("xcb", [3, 16, H, H], bf16, kind="ExternalInput")
    ry = nc.dram_tensor("ry", [128, 2, 112], bf16, kind="ExternalInput")
    rx = nc.dram_tensor("rx", [128, 2, CROP], bf16, kind="ExternalInput")
    offs = nc.dram_tensor("offs", [1, 4], i32, kind="ExternalInput")
    out = nc.dram_tensor("out", [3, 16, CROP, CROP], bf16, kind="ExternalOutput")

    with tile.TileContext(nc) as tc:
        with (
            tc.tile_pool(name="const", bufs=1) as constp,
            tc.tile_pool(name="xin", bufs=2) as xinp,
            tc.tile_pool(name="atp", bufs=4) as atp,
            tc.tile_pool(name="otp", bufs=6) as otp,
            tc.tile_pool(name="psa", bufs=4, space="PSUM") as psap,
            tc.tile_pool(name="pso", bufs=4, space="PSUM") as psop,
        ):
            ryt = constp.tile([128, 2, 112], bf16)
            rxt = constp.tile([128, 2, CROP], bf16)
            offs_sb = constp.tile([1, 4], i32)
            # offs first: the window-base reg_loads (and every window DMA
            # behind them) wait on it.  Consts ride the ACT queue.
            nc.sync.dma_start(out=offs_sb[:], in_=offs[:])
            nc.scalar.dma_start(out=ryt[:], in_=ry[:])
            nc.scalar.dma_start(out=rxt[:], in_=rx[:])

            # window bases -> SP sequencer registers.  Each dynamic DMA
            # consumes bounds-check register pairs, so the DMA count stays
            # small (~10).
            engines = [mybir.EngineType.SP]
            svs = []
            for k, hi in ((0, 160), (1, 160), (2, 31)):
                regs = nc.alloc_registers(f"offs{k}", engines)
                nc.regs_load(regs, offs_sb[0:1, k : k + 1])
                svs.append(nc.snap(regs, donate=True, min_val=0, max_val=hi))
            svy = svs[:2]
            svx = svs[2]

            # Per-channel window tiles [128 rows of half m, 16 frames, 256
            # cols] (512 B DRAM runs) from a bufs=2 pool: channel c+2's DMA
            # must wait for channel c's last stage-1 reader, which spreads
            # input transfers across the kernel instead of saturating the 16
            # DMA engines up front.  All input descriptor generation runs on
            # the SP HWDGE queue; output DMAs ride Pool's SWDGE so the ACT
            # queue does copies only.
            def issue_in(c):
                xw = [
                    xinp.tile([128, 16, WIN], bf16, name=f"xw{m}", tag=f"xw{m}")
                    for m in range(2)
                ]
                steps = ((slice(0, 4), slice(4, 16)) if c == 0 else (slice(0, 16),))
                for th in steps:
                    for m in range(2):
                        src = xcb[c, th, bass.ds(svy[m], 128), bass.ds(svx, WIN)]
                        nc.sync.dma_start(
                            out=xw[m][:, th, :],
                            in_=src.rearrange("t h w -> h t w"),
                        )
                return xw

            xw_c = {0: issue_in(0), 1: issue_in(1)}

            for c in range(3):
                if c + 1 < 3 and c + 1 not in xw_c:
                    xw_c[c + 1] = issue_in(c + 1)
                xw = xw_c.pop(c)

                psa_t = {}
                at_t = {}
                ot = None

                def stage1(t):
                    # psA free layout [wt, r, p(pad 128)]: A^T[wt*128+k, 2p+r].
                    # p is padded 112->128 so stage-2 lhsT tiles are full
                    # 128 columns -> Fast Weight Load (2x LDWEIGHTS).  The
                    # pad columns are never written (junk flows only into
                    # psO partitions 112:127, which are never copied out).
                    psa_t[t] = psap.tile([128, 2, 2, 128], f32, name="psa", tag="psa")
                    for m in range(2):
                        for wt in range(2):
                            nc.tensor.matmul(
                                psa_t[t][:, wt, :, 56 * m : 56 * m + 56],
                                lhsT=xw[m][:, t, wt * 128 : (wt + 1) * 128],
                                rhs=ryt[:, m, :],
                                start=True,
                                stop=True,
                            )

                def stage2(t):
                    psa = psa_t.pop(t)
                    at = atp.tile([128, 2, 2, 128], bf16, name="at", tag="at")
                    nc.vector.tensor_copy(at[:], psa[:])
                    at_t[t] = at
                    pso = psop.tile([128, 2, CROP], f32, name="pso", tag="pso")
                    for r in range(2):
                        for wt in range(2):
                            nc.tensor.matmul(
                                pso[:, r, :],
                                lhsT=at[:, wt, r, :],
                                rhs=rxt[:, wt, :],
                                start=(wt == 0),
                                stop=(wt == 1),
                            )
                    nc.scalar.copy(out=ot[:, t % 4, :, :], in_=pso[0:112, :, :])
                    if t % 4 == 3:
                        th = slice(t - 3, t + 1)
                        # alternate output groups across the SWDGE and ACT
                        # HWDGE queues: each SWDGE DMA is serialized behind
                        # its drain, so one queue alone has no slack vs the
                        # 4-frame production period; two queues double the
                        # out-chain throughput
                        oeng = nc.gpsimd if (c * 16 + t) % 8 == 3 else nc.scalar
                        oeng.dma_start(
                            out=out[c, th, :, :].rearrange(
                                "t (p r) j -> p t (r j)", p=112, r=2
                            ),
                            in_=ot[:, :, :, :].rearrange("p t r j -> p t (r j)"),
                        )

                for t in range(16):
                    if t % 4 == 0:
                        ot = otp.tile([112, 4, 2, CROP], bf16, name="ot", tag="ot")
                    if t == 0:
                        stage1(0)
                        stage1(1)
                    if t + 2 < 16:
                        stage1(t + 2)
                    stage2(t)
    _split_multi_waits(nc)
    return nc


def kernel(x, cam_views, resize, y_offset, x_offset):
    global _PROGRAM, LAST_RESULTS
    from concourse.bass_utils import run_bass_kernel_spmd

    x = np.asarray(x)
    cam_views = np.asarray(cam_views)
    resize = np.asarray(resize, dtype=np.float32)
    y_offset = np.asarray(y_offset, dtype=np.float32)
    x_offset = np.asarray(x_offset, dtype=np.float32)

    B = x.shape[0]
    assert x.shape == (8, 3, 16, H, H), x.shape

    # reference's clamp/floor in float32
    r = np.floor(np.clip(resize, np.float32(H), np.float32(RESIZE_MAX)))
    yo = np.floor(np.clip(y_offset, np.float32(0.0), r - np.float32(CROP)))
    xo = np.floor(np.clip(x_offset, np.float32(0.0), r - np.float32(CROP)))

    # output-row pair permutation: ry col c = r*56+p  <->  i-half row 2p+r
    perm = np.concatenate([np.arange(0, 112, 2), np.arange(1, 112, 2)])

    ry_v, rx_v, offs_v = [], [], []
    for v in range(r.shape[0]):
        y0, y1, wy = _coords(yo[v], r[v])
        x0, x1, wx = _coords(xo[v], r[v])
        ybase = [int(y0[0]), int(y0[112])]
        xbase = int(x0[0])
        for m in range(2):
            sl = slice(112 * m, 112 * (m + 1))
            assert y0[sl].min() >= ybase[m] and y1[sl].max() < ybase[m] + 128
        assert x0.min() >= xbase and x1.max() < xbase + WIN

        ry = np.zeros((128, 2, 112), dtype=np.float32)
        for m in range(2):
            sl = slice(112 * m, 112 * (m + 1))
            mh = _tap_matrix(y0[sl], y1[sl], wy[sl], ybase[m], 128, 112)
            ry[:, m, :] = mh[:, perm]
        rxm = _tap_matrix(x0, x1, wx, xbase, WIN, CROP)  # [256, 224]
        rx_p = rxm.reshape(2, 128, CROP).transpose(1, 0, 2)  # [128, wt, j]

        ry_v.append(np.ascontiguousarray(ry.astype(BF16)))
        rx_v.append(np.ascontiguousarray(rx_p.astype(BF16)))
        offs_v.append(
            np.array([[ybase[0], ybase[1], xbase, 0]], dtype=np.int32)
        )

    if _PROGRAM is None:
        _PROGRAM = _build_program()

    in_maps = []
    for b in range(B):
        v = int(cam_views[b])
        in_maps.append(
            {
                "xcb": np.ascontiguousarray(x[b].astype(BF16)),
                "ry": ry_v[v],
                "rx": rx_v[v],
                "offs": offs_v[v],
            }
        )

    res = run_bass_kernel_spmd(_PROGRAM, in_maps, list(range(B)), trace=TRACE)
    LAST_RESULTS = res
    return np.stack(
        [res.results[b]["out"].astype(np.float32) for b in range(B)], axis=0
    )


# revision 23
# speedup vs baseline: 1.2235x; 1.2235x over previous
"""Trainium2 Bass kernel for nn_CropPrompter.

Fused resize+crop bilinear sampling of video clips:
  x[8,3,16,512,512] --(per-clip crop geometry from cam_views/resize/offsets)-->
  out[8,3,16,224,224]

Strategy (pure data parallel, 1 clip per NeuronCore, 8 cores), v2:
  * All device compute in bf16 (PSUM accumulates f32).  Host converts x to
    bf16 (halves input DMA), packs the per-view bilinear interpolation
    matrices in bf16, and upcasts the bf16 device output back to f32.
    Measured end-to-end rel err ~2.9e-3 vs the f32 reference (gate 2e-2).
  * Banded stage 1: because resize >= H=512, the source-row span of any 112
    consecutive output rows is <= 114 < 128.  The device loads, per clip, two
    data-dependent 128-row x 256-col windows of each frame (dynamic-offset
    DMA; the row/col window bases ride in as an int32 `offs` input read into
    sequencer registers), so the row interpolation for each output half is a
    SINGLE k=128 matmul instead of a k=256 accumulation pair:
      A^T[w', i-half m] = win_m^T @ RyT_m      4 matmuls @ N=112 per frame
  * Stage 2 contracts the static 256-col window (2 k-tiles):
      out[i, j]        = A^T^T @ RxT           4 matmuls @ N=224 per frame
  * bf16 matmuls run 1 cycle/row at any N (f32r needs N>=256 for full rate),
    and full-128-col bf16 weights get Fast Weight Load; LDWEIGHTS pulls
    ahead of in-flight matmuls so weight loads hide behind streaming.
  * A^T's i axis is (r, p) pair-permuted (i = 2p+r, via host permutation of
    RyT's columns) so each output-store partition holds an i row pair ->
    896 B contiguous DRAM runs (>= 512 B: no DMA read-modify-write penalty).
"""

import numpy as np
import ml_dtypes

CROP = 224
H = 512
RESIZE_MAX = 1024
WIN = 256  # static col-window width (cols [xbase, xbase+256) cover all taps)

_PROGRAM = None
TRACE = False
LAST_RESULTS = None

BF16 = ml_dtypes.bfloat16


def _coords(off, rb):
    """Replicates reference._coords in numpy float32, op-for-op."""
    i = np.arange(CROP, dtype=np.float32)
    src = (np.float32(off) + i + np.float32(0.5)) * (np.float32(H) / np.float32(rb)) - np.float32(0.5)
    src = np.maximum(src, np.float32(0.0))
    i0 = np.clip(np.floor(src).astype(np.int32), 0, H - 1)
    i1 = np.minimum(i0 + 1, H - 1)
    w = src - i0.astype(np.float32)
    return i0, i1, w


def _tap_matrix(i0, i1, w, base, K, N):
    """[K, N] f32 M with M[k, n] = bilinear weight of source row (base+k)
    for output row n (of the N-slice whose taps i0/i1/w are passed)."""
    m = np.zeros((K, N), dtype=np.float32)
    idx = np.arange(N)
    np.add.at(m, (i0 - base, idx), np.float32(1.0) - w)
    np.add.at(m, (i1 - base, idx), w)
    return m


def _split_multi_waits(nc):
    """Walrus (kernel-dev pipeline) allows only one semaphore wait per
    instruction; hoist extra waits onto standalone EventSemaphore
    instructions inserted just before, on the same engine."""
    from concourse import mybir

    n = 0
    for fn in nc.m.functions:
        for bb in fn.blocks:
            out = []
            changed = False
            for inst in bb.instructions:
                si = getattr(inst, "sync_info", None)
                waits = list(si.on_wait) if si is not None and si.on_wait else []
                if len(waits) > 1:
                    for k, w in enumerate(waits[:-1]):
                        out.append(
                            mybir.InstEventSemaphore(
                                name=f"{inst.name}-w{k}",
                                ins=[],
                                outs=[],
                                engine=inst.engine,
                                sync_info=mybir.SyncInfo(on_wait=[w], on_update=[]),
                            )
                        )
                        n += 1
                    inst.sync_info = mybir.SyncInfo(
                        on_wait=[waits[-1]], on_update=list(si.on_update or [])
                    )
                    changed = True
                out.append(inst)
            if changed:
                bb.instructions = out
    return n


def _build_program():
    from concourse import bass, mybir, tile

    f32 = mybir.dt.float32
    bf16 = mybir.dt.bfloat16
    i32 = mybir.dt.int32

    nc = bass.Bass()
    xcb = nc.dram_tensor("xcb", [3, 16, H, H], bf16, kind="ExternalInput")
    ry = nc.dram_tensor("ry", [128, 2, 112], bf16, kind="ExternalInput")
    rx = nc.dram_tensor("rx", [128, 2, CROP], bf16, kind="ExternalInput")
    offs = nc.dram_tensor("offs", [1, 4], i32, kind="ExternalInput")
    out = nc.dram_tensor("out", [3, 16, CROP, CROP], bf16, kind="ExternalOutput")

    with tile.TileContext(nc) as tc:
        with (
            tc.tile_pool(name="const", bufs=1) as constp,
            tc.tile_pool(name="xin", bufs=2) as xinp,
            tc.tile_pool(name="atp", bufs=4) as atp,
            tc.tile_pool(name="otp", bufs=4) as otp,
            tc.tile_pool(name="psa", bufs=4, space="PSUM") as psap,
            tc.tile_pool(name="pso", bufs=4, space="PSUM") as psop,
        ):
            ryt = constp.tile([128, 2, 112], bf16)
            rxt = constp.tile([128, 2, CROP], bf16)
            offs_sb = constp.tile([1, 4], i32)
            # offs first: the window-base reg_loads (and every window DMA
            # behind them) wait on it.  Consts ride the ACT queue.
            nc.sync.dma_start(out=offs_sb[:], in_=offs[:])
            nc.scalar.dma_start(out=ryt[:], in_=ry[:])
            nc.scalar.dma_start(out=rxt[:], in_=rx[:])

            # window bases -> SP sequencer registers.  Each dynamic DMA
            # consumes bounds-check register pairs, so the DMA count stays
            # small (~10).
            engines = [mybir.EngineType.SP]
            svs = []
            for k, hi in ((0, 160), (1, 160), (2, 31)):
                regs = nc.alloc_registers(f"offs{k}", engines)
                nc.regs_load(regs, offs_sb[0:1, k : k + 1])
                svs.append(nc.snap(regs, donate=True, min_val=0, max_val=hi))
            svy = svs[:2]
            svx = svs[2]

            # Per-channel window tiles [128 rows of half m, 16 frames, 256
            # cols] (512 B DRAM runs) from a bufs=2 pool: channel c+2's DMA
            # must wait for channel c's last stage-1 reader, which spreads
            # input transfers across the kernel instead of saturating the 16
            # DMA engines up front.  All input descriptor generation runs on
            # the SP HWDGE queue; output DMAs ride Pool's SWDGE so the ACT
            # queue does copies only.
            def issue_in(c):
                xw = [
                    xinp.tile([128, 16, WIN], bf16, name=f"xw{m}", tag=f"xw{m}")
                    for m in range(2)
                ]
                steps = ((slice(0, 4), slice(4, 16)) if c == 0 else (slice(0, 16),))
                for th in steps:
                    for m in range(2):
                        src = xcb[c, th, bass.ds(svy[m], 128), bass.ds(svx, WIN)]
                        nc.sync.dma_start(
                            out=xw[m][:, th, :],
                            in_=src.rearrange("t h w -> h t w"),
                        )
                return xw

            xw_c = {0: issue_in(0), 1: issue_in(1)}

            for c in range(3):
                if c + 1 < 3 and c + 1 not in xw_c:
                    xw_c[c + 1] = issue_in(c + 1)
                xw = xw_c.pop(c)

                psa_t = {}
                at_t = {}
                ot = None

                def stage1(t):
                    # psA free layout [wt, r, p(pad 128)]: A^T[wt*128+k, 2p+r].
                    # p is padded 112->128 so stage-2 lhsT tiles are full
                    # 128 columns -> Fast Weight Load (2x LDWEIGHTS).  The
                    # pad columns are never written (junk flows only into
                    # psO partitions 112:127, which are never copied out).
                    psa_t[t] = psap.tile([128, 2, 2, 128], f32, name="psa", tag="psa")
                    for m in range(2):
                        for wt in range(2):
                            nc.tensor.matmul(
                                psa_t[t][:, wt, :, 56 * m : 56 * m + 56],
                                lhsT=xw[m][:, t, wt * 128 : (wt + 1) * 128],
                                rhs=ryt[:, m, :],
                                start=True,
                                stop=True,
                            )

                def stage2(t):
                    psa = psa_t.pop(t)
                    at = atp.tile([128, 2, 2, 128], bf16, name="at", tag="at")
                    nc.vector.tensor_copy(at[:], psa[:])
                    at_t[t] = at
                    pso = psop.tile([128, 2, CROP], f32, name="pso", tag="pso")
                    for r in range(2):
                        for wt in range(2):
                            nc.tensor.matmul(
                                pso[:, r, :],
                                lhsT=at[:, wt, r, :],
                                rhs=rxt[:, wt, :],
                                start=(wt == 0),
                                stop=(wt == 1),
                            )
                    nc.scalar.copy(out=ot[:, t % 4, :, :], in_=pso[0:112, :, :])
                    if t % 4 == 3:
                        th = slice(t - 3, t + 1)
                        nc.gpsimd.dma_start(
                            out=out[c, th, :, :].rearrange(
                                "t (p r) j -> p t (r j)", p=112, r=2
                            ),
                            in_=ot[:, :, :, :].rearrange("p t r j -> p t (r j)"),
                        )

                for t in range(16):
                    if t % 4 == 0:
                        ot = otp.tile([112, 4, 2, CROP], bf16, name="ot", tag="ot")
                    if t == 0:
                        stage1(0)
                        stage1(1)
                    if t + 2 < 16:
                        stage1(t + 2)
                    stage2(t)
    _split_multi_waits(nc)
    return nc


def kernel(x, cam_views, resize, y_offset, x_offset):
    global _PROGRAM, LAST_RESULTS
    from concourse.bass_utils import run_bass_kernel_spmd

    x = np.asarray(x)
    cam_views = np.asarray(cam_views)
    resize = np.asarray(resize, dtype=np.float32)
    y_offset = np.asarray(y_offset, dtype=np.float32)
    x_offset = np.asarray(x_offset, dtype=np.float32)

    B = x.shape[0]
    assert x.shape == (8, 3, 16, H, H), x.shape

    # reference's clamp/floor in float32
    r = np.floor(np.clip(resize, np.float32(H), np.float32(RESIZE_MAX)))
    yo = np.floor(np.clip(y_offset, np.float32(0.0), r - np.float32(CROP)))
    xo = np.floor(np.clip(x_offset, np.float32(0.0), r - np.float32(CROP)))

    # output-row pair permutation: ry col c = r*56+p  <->  i-half row 2p+r
    perm = np.concatenate([np.arange(0, 112, 2), np.arange(1, 112, 2)])

    ry_v, rx_v, offs_v = [], [], []
    for v in range(r.shape[0]):
        y0, y1, wy = _coords(yo[v], r[v])
        x0, x1, wx = _coords(xo[v], r[v])
        ybase = [int(y0[0]), int(y0[112])]
        xbase = int(x0[0])
        for m in range(2):
            sl = slice(112 * m, 112 * (m + 1))
            assert y0[sl].min() >= ybase[m] and y1[sl].max() < ybase[m] + 128
        assert x0.min() >= xbase and x1.max() < xbase + WIN

        ry = np.zeros((128, 2, 112), dtype=np.float32)
        for m in range(2):
            sl = slice(112 * m, 112 * (m + 1))
            mh = _tap_matrix(y0[sl], y1[sl], wy[sl], ybase[m], 128, 112)
            ry[:, m, :] = mh[:, perm]
        rxm = _tap_matrix(x0, x1, wx, xbase, WIN, CROP)  # [256, 224]
        rx_p = rxm.reshape(2, 128, CROP).transpose(1, 0, 2)  # [128, wt, j]

        ry_v.append(np.ascontiguousarray(ry.astype(BF16)))
        rx_v.append(np.ascontiguousarray(rx_p.astype(BF16)))
        offs_v.append(
            np.array([[ybase[0], ybase[1], xbase, 0]], dtype=np.int32)
        )

    if _PROGRAM is None:
        _PROGRAM = _build_program()

    in_maps = []
    for b in range(B):
        v = int(cam_views[b])
        in_maps.append(
            {
                "xcb": np.ascontiguousarray(x[b].astype(BF16)),
                "ry": ry_v[v],
                "rx": rx_v[v],
                "offs": offs_v[v],
            }
        )

    res = run_bass_kernel_spmd(_PROGRAM, in_maps, list(range(B)), trace=TRACE)
    LAST_RESULTS = res
    return np.stack(
        [res.results[b]["out"].astype(np.float32) for b in range(B)], axis=0
    )
